# revision 34
# baseline (speedup 1.0000x reference)
"""Distributed causal single-head attention for 8 Trainium2 NeuronCores.

Problem: x [B=4, T=4096, E=1024] f32; Wq/Wk/Wv [E, H=64] f32.
out[b] = softmax(causal(q k^T / sqrt(H))) v,  q/k/v = x[b] @ W.

Sharding: core = (batch b = core//2, parity o = core%2). Each core computes
the output rows of the interleaved 512-row chunks {2J+o : J=0..3} of batch b.
The host ships x[b]^T (bf16, tile-blocked for contiguous DMA) with columns
permuted to [own-chunks | partner-chunks] so all 8 cores run one identical
(SPMD) instruction stream; parity enters only through data (a scalar mask
input). Causal masks are generated on-device.

On this stack, externally-supplied DRAM buffers stream ~10x slower than
SBUF-resident data, so all of x (8 MB bf16/core) is staged into SBUF once
up front on the sync HWDGE queue; the compute body never touches external
DRAM again except the output store. The body itself is balanced across
three ~70 us engine streams (ScalarE exp / PE matmul / DVE evacuations):

  - q/k/v projections as bf16 matmuls (qk-pack [Wq|Wk], kv-pack [Wk|Wv]);
    own-block v is produced directly in natural [k,h] layout (stationary =
    x^T tile, moving = Wv), partner-block v^T goes through packed PE
    transposes (identity matmul, 4 tiles per PSUM bank, one evacuation
    copy per bank),
  - scoresT[k,q] via row-packed K=64 matmul pairs - the q/k row
    duplication places the pair at PE row offsets 0/64, which the
    hardware runs concurrently (inferred tile_position row tiling),
  - exp on ScalarE (PSUM f32 -> SBUF bf16, the throughput floor at
    ~880 ns per [128,512] tile), causal/parity masks multiplied in on DVE,
  - PV as bf16 matmuls with an extra ones-row in v_nat producing softmax
    denominators for free,
  - normalization via f32r reciprocal + K=1 broadcast matmul.

The attention phase is a flat software pipeline over all (chunk, k-tile
pair) steps: score matmuls are issued `depth` pairs ahead of their
exp->mask->PV consumers, score PSUM tiles rotate across three pools (six
banks in flight), and each chunk's normalization is deferred past the
next chunk's issue - the in-order PE otherwise head-of-line blocks on
every cross-engine hop. PSUM tiles in the projection phase rotate across
the same three pools.
"""

import sys

sys.path.insert(0, "/opt/trn_rl_repo")

import numpy as np

import concourse.bass as bass  # noqa: F401
import concourse.tile as tile
from concourse import bacc, mybir
from concourse.bass_utils import run_bass_kernel_spmd

F32 = mybir.dt.float32
F32R = mybir.dt.float32r
BF16 = mybir.dt.bfloat16

B, T, E, H = 4, 4096, 1024, 64
N_CORES = 8
SCALE = float(H) ** -0.5


def build_attention(Eb=E, Tg=T, CH=512, n_loop=1, n_cores=N_CORES,
                    do_proj=True, do_attn=True, do_vnat=True,
                    depth=4, defer_fin=True, ex_bufs=4, unrolled=False,
                    direct_v=True, gp_copies=False, out_mode='batched'):
    """Per-core inputs:
    xb [2*NCH, 128, NE*CH] bf16  (xb[b,p,e*CH+c] = x_local[b*CH+c, e*128+p]),
    w  [NE, 128, 192] bf16       ([Wq | Wk | Wv] row-tiled),
    pm [128, 1] f32, ones [1, H] f32r  ->  outT [H, Tl] bf16.
    """
    Tl = Tg // 2
    TPC = CH // 128          # k-tiles per chunk
    NCH = Tl // CH           # local q-chunks per core
    NTl = Tl // 128          # own k-tiles
    NTg = 2 * NTl            # all k-tiles (own + partner)
    NE = Eb // 128           # contraction tiles
    NB = 2 * NCH             # column blocks (own + partner)
    VW = 128                 # padded v row width (xbar-safe)

    nc = bacc.Bacc("TRN2", target_bir_lowering=False, debug=False,
                   num_devices=n_cores)
    xb_ext = nc.dram_tensor("xb", [NB, 128, NE * CH], BF16, kind="ExternalInput")
    w_ext = nc.dram_tensor("w", [NE, 128, 192], BF16, kind="ExternalInput")
    pm_ext = nc.dram_tensor("pm", [128, 1], F32, kind="ExternalInput")
    ones_ext = nc.dram_tensor("ones", [1, H], F32R, kind="ExternalInput")
    outT_ext = nc.dram_tensor("outT", [H, Tl], BF16, kind="ExternalOutput")

    with tile.TileContext(nc) as tc:
        with (
            tc.tile_pool(name="const", bufs=1) as cpool,
            tc.tile_pool(name="big", bufs=1) as bigpool,
            tc.tile_pool(name="work", bufs=4) as wpool,
            tc.tile_pool(name="exp", bufs=ex_bufs) as expool,
            tc.tile_pool(name="psS", bufs=3, space="PSUM") as psS,
            tc.tile_pool(name="psO", bufs=2, space="PSUM") as psO,
        ):
            # ---- constants ----
            w = cpool.tile([128, NE, 192], BF16, tag="w")
            tri = cpool.tile([128, TPC, CH], BF16, tag="tri")
            pm = cpool.tile([128, 1], F32, tag="pm")
            ones64 = cpool.tile([1, H], F32R, tag="ones64")
            for e in range(NE):
                nc.gpsimd.dma_start(out=w[:, e, :], in_=w_ext.ap()[e])
            nc.gpsimd.dma_start(out=pm[:, :], in_=pm_ext.ap())
            nc.gpsimd.dma_start(out=ones64[:, :], in_=ones_ext.ap())
            # tri[i][p, f] = 1.0 if 128*i + p <= f else 0.0
            for i in range(TPC):
                nc.gpsimd.memset(tri[:, i, :], 0.0)
                nc.gpsimd.affine_select(
                    out=tri[:, i, :], in_=tri[:, i, :],
                    compare_op=mybir.AluOpType.is_gt, fill=1.0,
                    base=128 * i, pattern=[[-1, CH]], channel_multiplier=1)
            ident = cpool.tile([128, 128], F32, tag="ident")
            from concourse.masks import make_identity
            make_identity(nc, ident[:, :])

            # ---- x resident in SBUF (loaded once, before the loop) ----
            # External-input DRAM in this environment streams ~10x slower
            # than SBUF-resident data, so stage all of x into SBUF up front
            # on the sync HWDGE queue; the compute loop never touches
            # external DRAM again except the output store.
            xsb = cpool.tile([128, NB, NE, CH], BF16, tag="xsb")
            for b in range(NB):
                nc.sync.dma_start(
                    out=xsb[:, b, :, :],
                    in_=xb_ext.ap()[b].rearrange("p (e c) -> p e c", e=NE))

            # persistent across loop iterations; fully rewritten each pass
            qdup = bigpool.tile([128, Tl], BF16, tag="qdup")
            kdup = bigpool.tile([128, Tg], BF16, tag="kdup")
            vT_sb = bigpool.tile([H, Tl], F32, tag="vT_sb")
            vT_own = bigpool.tile([H, Tl], F32, tag="vT_own")
            v_nat = bigpool.tile([128, NTg, VW], BF16, tag="v_nat")
            o_stage = bigpool.tile([H, Tl], BF16, tag="o_stage")
            # zero cols once so the padded 128-wide PV stationary (for FWL)
            # multiplies garbage-free; ones column = softmax denominator
            nc.vector.memset(v_nat[:, :, :], 0.0)
            nc.vector.memset(v_nat[:, :, H:H + 1], 1.0)
            for _i in range(ex_bufs):
                exz = expool.tile([128, 2 * CH], BF16, tag="ex", name="exz")
                nc.vector.memset(exz[:, :], 0.0)

            def body(_iv=0, unroll=1):
                _pp = [0, None]

                def proj_ps(nm):
                    if _pp[0] % 2 == 0:
                        _pp[1] = psS.tile([128, 2 * CH], F32, tag="s",
                                          name=nm)
                        half = _pp[1][:, 0:CH]
                    else:
                        half = _pp[1][:, CH:2 * CH]
                    _pp[0] += 1
                    return half

                # ---- projections ----
                if not do_proj:
                    nc.vector.memset(qdup[:, :], 0.0)
                    nc.vector.memset(kdup[:, :], 0.0)
                    nc.vector.memset(vT_sb[0:64, :], 0.0)
                # blocks in pairs: two qk/kv chains fill the two banks of
                # one PSUM double tile, evacuated with single fused copies
                for b0 in range(0, NB if do_proj else 0, 2):
                    own = b0 < NCH
                    S = psS.tile([128, 2 * CH], F32, tag="s", name="Sp")
                    for h, b in ((0, b0), (1, b0 + 1)):
                        woff = 0 if own else 64  # [Wq|Wk] or [Wk|Wv]
                        for e in range(NE):
                            nc.tensor.matmul(S[:, h * CH:h * CH + CH],
                                             w[:, e, woff:woff + 128],
                                             xsb[:, b, e, :],
                                             start=(e == 0),
                                             stop=(e == NE - 1))
                    cols2 = slice(b0 * CH, (b0 + 2) * CH)
                    if own:
                        nc.vector.tensor_copy(qdup[0:64, cols2], S[0:64, :])
                        nc.vector.tensor_copy(qdup[64:128, cols2], S[0:64, :])
                        nc.vector.tensor_copy(kdup[0:64, cols2], S[64:128, :])
                        nc.vector.tensor_copy(kdup[64:128, cols2],
                                              S[64:128, :])
                        # v in natural [k, h] layout directly: stationary is
                        # the x^T tile (contraction = e), moving is Wv
                        V = psS.tile([128, 2 * CH], F32, tag="s", name="Vp")
                        for h, b in ((0, b0), (1, b0 + 1)):
                            for j in range(TPC):
                                for e in range(NE):
                                    nc.tensor.matmul(
                                        V[:, h * CH + j * 128:
                                          h * CH + j * 128 + H],
                                        xsb[:, b, e, j * 128:(j + 1) * 128],
                                        w[:, e, 128:192],
                                        start=(e == 0), stop=(e == NE - 1))
                        nc.vector.tensor_copy(
                            v_nat[:, b0 * TPC:(b0 + 2) * TPC, 0:H],
                            V[:, 0:2 * CH].rearrange(
                                "p (h t c) -> p (h t) c", h=2, t=TPC)[
                                :, :, 0:H])
                    else:
                        nc.vector.tensor_copy(kdup[0:64, cols2], S[0:64, :])
                        nc.vector.tensor_copy(kdup[64:128, cols2], S[0:64, :])
                        pcols2 = slice((b0 - NCH) * CH, (b0 - NCH + 2) * CH)
                        nc.vector.tensor_copy(vT_sb[0:64, pcols2],
                                              S[64:128, :])

                # v natural (+ones row): per-tile PE transpose [65,128]->[128,65]
                # through PSUM (identity matmul) - far cheaper than the xbar
                # DMA-transpose path, and it pipelines with the projections
                # on the same engine.
                srcs = [(NTl, vT_sb)] if direct_v else [(NTl, vT_sb),
                                                         (0, vT_own)]
                for t0, vsrc in (srcs if do_vnat else []):
                    for t4 in range(NTl // TPC):
                        tps = proj_ps("tps")
                        for j in range(TPC):
                            t = t4 * TPC + j
                            nc.tensor.transpose(
                                tps[:, j * H:(j + 1) * H],
                                vsrc[0:H, t * 128:(t + 1) * 128],
                                ident[0:H, 0:H])
                        nc.vector.tensor_copy(
                            v_nat[:, t0 + t4 * TPC:t0 + (t4 + 1) * TPC, 0:H],
                            tps[:, 0:TPC * H].rearrange("p (t c) -> p t c",
                                                        t=TPC))

                # ---- attention ----
                if not do_attn:
                    nc.sync.dma_start(out=outT_ext.ap()[:, 0:Tl],
                                      in_=qdup[0:H, 0:Tl])
                # Flat software pipeline over all (J, k-tile-pair) steps.
                # The PE is in-order, so QK score matmuls are issued DEPTH
                # pairs ahead of the exp->mask->PV consumer chain, and each
                # chunk's normalization is deferred past the next chunk's QK
                # issue; otherwise every cross-engine hop head-of-line
                # blocks the PE.
                pairs = []   # (J, idxA, idxB, n_J)
                ent_of_J = {}
                for J in range(NCH if do_attn else 0):
                    entries = []
                    for c in range(J):
                        for i in range(TPC):
                            entries.append((TPC * c + i, None))
                    for c in range(J):
                        for i in range(TPC):
                            entries.append((NTl + TPC * c + i, None))
                    for i in range(TPC):
                        entries.append((TPC * J + i, ("tri", i)))
                    for i in range(TPC):
                        entries.append((NTl + TPC * J + i, ("pm", 0)))
                    ent_of_J[J] = entries
                    for p in range(len(entries) // 2):
                        pairs.append((J, 2 * p, 2 * p + 1, len(entries)))

                G = len(pairs)
                DEPTH = depth
                score_tiles = {}
                o_of_J = {}

                def issue_qk(g):
                    J, ia, ib, _n = pairs[g]
                    qb = slice(J * CH, (J + 1) * CH)
                    t0 = ent_of_J[J][ia][0]
                    t1 = ent_of_J[J][ib][0]
                    S = psS.tile([128, 2 * CH], F32, tag="s", name="S")
                    c0 = slice(t0 * 128, t0 * 128 + 128)
                    c1 = slice(t1 * 128, t1 * 128 + 128)
                    nc.tensor.matmul(S[:, 0:CH], kdup[0:64, c0],
                                     qdup[0:64, qb], start=True, stop=True)
                    nc.tensor.matmul(S[:, CH:2 * CH], kdup[64:128, c1],
                                     qdup[64:128, qb], start=True, stop=True)
                    score_tiles[g] = S

                def do_pair(g):
                    J, ia, ib, n = pairs[g]
                    S = score_tiles.pop(g)
                    (t0, mask_a) = ent_of_J[J][ia]
                    (t1, mask_b) = ent_of_J[J][ib]
                    ex = expool.tile([128, 2 * CH], BF16, tag="ex")
                    # one exp over both banks of the pair - amortizes the
                    # fixed ScalarE instruction cost; the steep diagonal
                    # pair skips its leading all-masked columns
                    c0 = 2 * 128 if mask_a == ("tri", TPC - 2) else 0
                    nc.scalar.activation(ex[:, c0:], S[:, c0:],
                                         mybir.ActivationFunctionType.Exp,
                                         scale=SCALE)
                    if mask_a is not None and mask_a[0] == "tri":
                        i0 = mask_a[1]
                        if mask_b == ("tri", i0 + 1):
                            nc.vector.tensor_mul(
                                ex[:], ex[:],
                                tri[:, i0:i0 + 2, :].rearrange(
                                    "p t c -> p (t c)"))
                        else:
                            nc.vector.tensor_mul(ex[:, 0:CH], ex[:, 0:CH],
                                                 tri[:, i0, :])
                    elif mask_a is not None:
                        nc.vector.tensor_scalar_mul(ex[:], ex[:], pm[:, :])
                    if mask_b is not None and mask_b[0] == "tri" and not (
                            mask_a is not None and mask_a[0] == "tri"
                            and mask_b == ("tri", mask_a[1] + 1)):
                        nc.vector.tensor_mul(ex[:, CH:2 * CH],
                                             ex[:, CH:2 * CH],
                                             tri[:, mask_b[1], :])
                    if ia == 0:
                        o_of_J[J] = psO.tile([128, CH], F32, tag="o",
                                             name="o_ps")
                    o_ps = o_of_J[J]
                    nc.tensor.matmul(o_ps[0:H + 1, :], v_nat[:, t0, 0:H + 1],
                                     ex[:, 0:CH],
                                     start=(ia == 0), stop=False)
                    nc.tensor.matmul(o_ps[0:H + 1, :], v_nat[:, t1, 0:H + 1],
                                     ex[:, CH:2 * CH],
                                     start=False, stop=(ib == n - 1))

                def finish_J(J):
                    qb = slice(J * CH, (J + 1) * CH)
                    o_ps = o_of_J.pop(J)
                    recip = wpool.tile([1, CH], F32R, tag="recip")
                    with nc.allow_low_precision(reason="f32r recip for norm"):
                        nc.vector.reciprocal(recip[:], o_ps[H:H + 1, :])
                    o_sb0 = wpool.tile([H, CH], F32, tag="osb0")
                    nc.vector.tensor_copy(o_sb0[:], o_ps[0:H, :])
                    rb_ps = psO.tile([128, CH], F32, tag="o", name="rb_ps")
                    nc.tensor.matmul(rb_ps[0:H, :], ones64[:], recip[:],
                                     start=True, stop=True)
                    nc.vector.tensor_mul(o_stage[:, qb], o_sb0[:],
                                         rb_ps[0:H, :])

                for g in range(min(DEPTH, G)):
                    issue_qk(g)
                pend_fin = None
                for g in range(G):
                    if g + DEPTH < G:
                        issue_qk(g + DEPTH)
                    if pend_fin is not None:
                        finish_J(pend_fin)
                        pend_fin = None
                    J, ia, ib, n = pairs[g]
                    do_pair(g)
                    if ib == n - 1:
                        if defer_fin:
                            pend_fin = J
                        else:
                            finish_J(J)
                if pend_fin is not None:
                    finish_J(pend_fin)
                if do_attn and out_mode == 'batched':
                    nc.sync.dma_start(out=outT_ext.ap()[:, :], in_=o_stage[:])

            if n_loop == 1:
                body()
                if out_mode == 'none':
                    nc.sync.dma_start(out=outT_ext.ap()[:, :], in_=o_stage[:])
            elif unrolled:
                for _i in range(n_loop):
                    body(_i)
            else:
                with tc.For_i(0, n_loop, 1) as iv:
                    body(iv)
                if out_mode == 'none':
                    nc.sync.dma_start(out=outT_ext.ap()[:, :], in_=o_stage[:])

    nc.compile()
    return nc


# ---------------- host-side shard / unshard ----------------

def make_in_maps(x, Wq, Wk, Wv, Tg=T, CH=512):
    import ml_dtypes
    Tl = Tg // 2
    NCH = Tl // CH
    NB = 2 * NCH
    NE = np.asarray(Wq).shape[0] // 128
    x = np.asarray(x)
    w_all = np.concatenate([np.asarray(Wq), np.asarray(Wk), np.asarray(Wv)],
                           axis=1).astype(ml_dtypes.bfloat16)     # [E, 192]
    w_tiled = np.ascontiguousarray(w_all.reshape(NE, 128, 192))

    in_maps = []
    for core in range(N_CORES):
        b, o = core // 2, core % 2
        own = [2 * J + o for J in range(NCH)]
        par = [2 * J + (1 - o) for J in range(NCH)]
        xl = np.concatenate([x[b, g * CH:(g + 1) * CH, :] for g in own + par],
                            axis=0)                               # [Tg, E]
        # xb[blk, p, e*CH + c] = xl[blk*CH + c, e*128 + p]
        xb = xl.reshape(NB, CH, NE, 128).transpose(0, 3, 2, 1)
        xb = np.ascontiguousarray(
            xb.reshape(NB, 128, NE * CH).astype(ml_dtypes.bfloat16))
        pmv = np.full((128, 1), 1.0 if o == 1 else 0.0, np.float32)
        in_maps.append({"xb": xb, "w": w_tiled, "pm": pmv,
                        "ones": np.ones((1, H), np.float32)})
    return in_maps


def unshard_out(results, Tg=T, CH=512):
    Tl = Tg // 2
    NCH = Tl // CH
    out = np.zeros((B, Tg, H), np.float32)
    for core in range(N_CORES):
        b, o = core // 2, core % 2
        outT = np.asarray(results[core]["outT"]).astype(np.float32)
        for J in range(NCH):
            g = 2 * J + o
            out[b, g * CH:(g + 1) * CH, :] = outT[:, J * CH:(J + 1) * CH].T
    return out


_cached_nc = None


def kernel(x, Wq, Wk, Wv):
    global _cached_nc
    if _cached_nc is None:
        _cached_nc = build_attention()
    in_maps = make_in_maps(x, Wq, Wk, Wv)
    res = run_bass_kernel_spmd(_cached_nc, in_maps, core_ids=list(range(N_CORES)))
    return unshard_out(res.results)



# revision 35
# speedup vs baseline: 1.3066x; 1.3066x over previous
"""Distributed causal single-head attention for 8 Trainium2 NeuronCores.

Problem: x [B=4, T=4096, E=1024] f32; Wq/Wk/Wv [E, H=64] f32.
out[b] = softmax(causal(q k^T / sqrt(H))) v,  q/k/v = x[b] @ W.

Sharding: core = (batch b = core//2, parity o = core%2). Each core computes
the output rows of the interleaved 512-row chunks {2J+o : J=0..3} of batch b.
The host ships x[b]^T (bf16, tile-blocked for contiguous DMA) with columns
permuted to [own-chunks | partner-chunks] so all 8 cores run one identical
(SPMD) instruction stream; parity enters only through data (a scalar mask
input). Causal masks are generated on-device.

On this stack, externally-supplied DRAM buffers stream ~10x slower than
SBUF-resident data, so all of x (8 MB bf16/core) is staged into SBUF once
up front on the sync HWDGE queue; the compute body never touches external
DRAM again except the output store. The body itself is balanced across
three ~70 us engine streams (ScalarE exp / PE matmul / DVE evacuations):

  - q/k/v projections as bf16 matmuls (qk-pack [Wq|Wk], kv-pack [Wk|Wv]);
    own-block v is produced directly in natural [k,h] layout (stationary =
    x^T tile, moving = Wv), partner-block v^T goes through packed PE
    transposes (identity matmul, 4 tiles per PSUM bank, one evacuation
    copy per bank),
  - scoresT[k,q] via row-packed K=64 matmul pairs - the q/k row
    duplication places the pair at PE row offsets 0/64, which the
    hardware runs concurrently (inferred tile_position row tiling),
  - exp on ScalarE (PSUM f32 -> SBUF bf16, the throughput floor at
    ~880 ns per [128,512] tile), causal/parity masks multiplied in on DVE,
  - PV as bf16 matmuls with an extra ones-row in v_nat producing softmax
    denominators for free,
  - normalization via f32r reciprocal + K=1 broadcast matmul.

The attention phase is a flat software pipeline over all (chunk, k-tile
pair) steps: score matmuls are issued `depth` pairs ahead of their
exp->mask->PV consumers, score PSUM tiles rotate across three pools (six
banks in flight), and each chunk's normalization is deferred past the
next chunk's issue - the in-order PE otherwise head-of-line blocks on
every cross-engine hop. PSUM tiles in the projection phase rotate across
the same three pools.
"""

import sys

sys.path.insert(0, "/opt/trn_rl_repo")

import numpy as np

import concourse.bass as bass  # noqa: F401
import concourse.tile as tile
from concourse import bacc, mybir
from concourse.bass_utils import run_bass_kernel_spmd

F32 = mybir.dt.float32
F32R = mybir.dt.float32r
BF16 = mybir.dt.bfloat16

B, T, E, H = 4, 4096, 1024, 64
N_CORES = 8
SCALE = float(H) ** -0.5


def build_attention(Eb=E, Tg=T, CH=512, n_loop=1, n_cores=N_CORES,
                    do_proj=True, do_attn=True, do_vnat=True,
                    depth=4, defer_fin=True, ex_bufs=4, unrolled=False,
                    direct_v=True, gp_copies=False, out_mode='batched'):
    """Per-core inputs:
    xb [2*NCH, 128, NE*CH] bf16  (xb[b,p,e*CH+c] = x_local[b*CH+c, e*128+p]),
    w  [NE, 128, 192] bf16       ([Wq | Wk | Wv] row-tiled),
    pm [128, 1] f32, ones [1, H] f32r  ->  outT [H, Tl] bf16.
    """
    Tl = Tg // 2
    TPC = CH // 128          # k-tiles per chunk
    NCH = Tl // CH           # local q-chunks per core
    NTl = Tl // 128          # own k-tiles
    NTg = 2 * NTl            # all k-tiles (own + partner)
    NE = Eb // 128           # contraction tiles
    NB = 2 * NCH             # column blocks (own + partner)
    VW = 128                 # padded v row width (xbar-safe)

    nc = bacc.Bacc("TRN2", target_bir_lowering=False, debug=False,
                   num_devices=n_cores)
    xb_ext = nc.dram_tensor("xb", [NB, 128, NE * CH], BF16, kind="ExternalInput")
    w_ext = nc.dram_tensor("w", [NE, 128, 192], BF16, kind="ExternalInput")
    pm_ext = nc.dram_tensor("pm", [128, 1], F32, kind="ExternalInput")
    ones_ext = nc.dram_tensor("ones", [1, H], F32R, kind="ExternalInput")
    outT_ext = nc.dram_tensor("outT", [H, Tl], BF16, kind="ExternalOutput")

    with tile.TileContext(nc) as tc:
        with (
            tc.tile_pool(name="const", bufs=1) as cpool,
            tc.tile_pool(name="big", bufs=1) as bigpool,
            tc.tile_pool(name="work", bufs=4) as wpool,
            tc.tile_pool(name="exp", bufs=ex_bufs) as expool,
            tc.tile_pool(name="psS", bufs=3, space="PSUM") as psS,
            tc.tile_pool(name="psO", bufs=2, space="PSUM") as psO,
        ):
            # ---- constants ----
            w = cpool.tile([128, NE, 192], BF16, tag="w")
            tri = cpool.tile([128, TPC, CH], BF16, tag="tri")
            pm = cpool.tile([128, 1], F32, tag="pm")
            ones64 = cpool.tile([1, H], F32R, tag="ones64")
            for e in range(NE):
                nc.gpsimd.dma_start(out=w[:, e, :], in_=w_ext.ap()[e])
            nc.gpsimd.dma_start(out=pm[:, :], in_=pm_ext.ap())
            nc.gpsimd.dma_start(out=ones64[:, :], in_=ones_ext.ap())
            # tri slot s holds the mask of diag tile i = s^1 (pairs are
            # emitted steeper-tile-first so the all-masked region is a
            # contiguous prefix the exp can skip; slot-swapping keeps the
            # fused two-tile mask multiply contiguous): mask[i][p, f] =
            # 1.0 if 128*i + p <= f else 0.0
            for i in range(TPC):
                slot = i ^ 1
                nc.gpsimd.memset(tri[:, slot, :], 0.0)
                nc.gpsimd.affine_select(
                    out=tri[:, slot, :], in_=tri[:, slot, :],
                    compare_op=mybir.AluOpType.is_gt, fill=1.0,
                    base=128 * i, pattern=[[-1, CH]], channel_multiplier=1)
            ident = cpool.tile([128, 128], F32, tag="ident")
            from concourse.masks import make_identity
            make_identity(nc, ident[:, :])

            # ---- x resident in SBUF (loaded once, before the loop) ----
            # External-input DRAM in this environment streams ~10x slower
            # than SBUF-resident data, so stage all of x into SBUF up front
            # on the sync HWDGE queue; the compute loop never touches
            # external DRAM again except the output store.
            xsb = cpool.tile([128, NB, NE, CH], BF16, tag="xsb")
            for b in range(NB):
                nc.sync.dma_start(
                    out=xsb[:, b, :, :],
                    in_=xb_ext.ap()[b].rearrange("p (e c) -> p e c", e=NE))

            # persistent across loop iterations; fully rewritten each pass
            qdup = bigpool.tile([128, Tl], BF16, tag="qdup")
            kdup = bigpool.tile([128, Tg], BF16, tag="kdup")
            vT_sb = bigpool.tile([H, Tl], F32, tag="vT_sb")
            vT_own = bigpool.tile([H, Tl], F32, tag="vT_own")
            v_nat = bigpool.tile([128, NTg, VW], BF16, tag="v_nat")
            o_stage = bigpool.tile([H, Tl], BF16, tag="o_stage")
            # zero cols once so the padded 128-wide PV stationary (for FWL)
            # multiplies garbage-free; ones column = softmax denominator
            nc.vector.memset(v_nat[:, :, :], 0.0)
            nc.vector.memset(v_nat[:, :, H:H + 1], 1.0)
            for _i in range(ex_bufs):
                exz = expool.tile([128, 2 * CH], BF16, tag="ex", name="exz")
                nc.vector.memset(exz[:, :], 0.0)

            def body(_iv=0, unroll=1):
                _pp = [0, None]

                def proj_ps(nm):
                    if _pp[0] % 2 == 0:
                        _pp[1] = psS.tile([128, 2 * CH], F32, tag="s",
                                          name=nm)
                        half = _pp[1][:, 0:CH]
                    else:
                        half = _pp[1][:, CH:2 * CH]
                    _pp[0] += 1
                    return half

                # ---- projections ----
                if not do_proj:
                    nc.vector.memset(qdup[:, :], 0.0)
                    nc.vector.memset(kdup[:, :], 0.0)
                    nc.vector.memset(vT_sb[0:64, :], 0.0)
                # blocks in pairs: two qk/kv chains fill the two banks of
                # one PSUM double tile, evacuated with single fused copies
                for b0 in range(0, NB if do_proj else 0, 2):
                    own = b0 < NCH
                    S = psS.tile([128, 2 * CH], F32, tag="s", name="Sp")
                    for h, b in ((0, b0), (1, b0 + 1)):
                        woff = 0 if own else 64  # [Wq|Wk] or [Wk|Wv]
                        for e in range(NE):
                            nc.tensor.matmul(S[:, h * CH:h * CH + CH],
                                             w[:, e, woff:woff + 128],
                                             xsb[:, b, e, :],
                                             start=(e == 0),
                                             stop=(e == NE - 1))
                    cols2 = slice(b0 * CH, (b0 + 2) * CH)
                    if own:
                        nc.vector.tensor_copy(qdup[0:64, cols2], S[0:64, :])
                        nc.vector.tensor_copy(qdup[64:128, cols2], S[0:64, :])
                        nc.vector.tensor_copy(kdup[0:64, cols2], S[64:128, :])
                        nc.vector.tensor_copy(kdup[64:128, cols2],
                                              S[64:128, :])
                        # v in natural [k, h] layout directly: stationary is
                        # the x^T tile (contraction = e), moving is Wv
                        V = psS.tile([128, 2 * CH], F32, tag="s", name="Vp")
                        for h, b in ((0, b0), (1, b0 + 1)):
                            for j in range(TPC):
                                for e in range(NE):
                                    nc.tensor.matmul(
                                        V[:, h * CH + j * 128:
                                          h * CH + j * 128 + H],
                                        xsb[:, b, e, j * 128:(j + 1) * 128],
                                        w[:, e, 128:192],
                                        start=(e == 0), stop=(e == NE - 1))
                        nc.vector.tensor_copy(
                            v_nat[:, b0 * TPC:(b0 + 2) * TPC, 0:H],
                            V[:, 0:2 * CH].rearrange(
                                "p (h t c) -> p (h t) c", h=2, t=TPC)[
                                :, :, 0:H])
                    else:
                        nc.vector.tensor_copy(kdup[0:64, cols2], S[0:64, :])
                        nc.vector.tensor_copy(kdup[64:128, cols2], S[0:64, :])
                        pcols2 = slice((b0 - NCH) * CH, (b0 - NCH + 2) * CH)
                        nc.vector.tensor_copy(vT_sb[0:64, pcols2],
                                              S[64:128, :])

                # v natural (+ones row): per-tile PE transpose [65,128]->[128,65]
                # through PSUM (identity matmul) - far cheaper than the xbar
                # DMA-transpose path, and it pipelines with the projections
                # on the same engine.
                srcs = [(NTl, vT_sb)] if direct_v else [(NTl, vT_sb),
                                                         (0, vT_own)]
                for t0, vsrc in (srcs if do_vnat else []):
                    for t4 in range(NTl // TPC):
                        tps = proj_ps("tps")
                        for j in range(TPC):
                            t = t4 * TPC + j
                            nc.tensor.transpose(
                                tps[:, j * H:(j + 1) * H],
                                vsrc[0:H, t * 128:(t + 1) * 128],
                                ident[0:H, 0:H])
                        nc.vector.tensor_copy(
                            v_nat[:, t0 + t4 * TPC:t0 + (t4 + 1) * TPC, 0:H],
                            tps[:, 0:TPC * H].rearrange("p (t c) -> p t c",
                                                        t=TPC))

                # ---- attention ----
                if not do_attn:
                    nc.sync.dma_start(out=outT_ext.ap()[:, 0:Tl],
                                      in_=qdup[0:H, 0:Tl])
                # Flat software pipeline over all (J, k-tile-pair) steps.
                # The PE is in-order, so QK score matmuls are issued DEPTH
                # pairs ahead of the exp->mask->PV consumer chain, and each
                # chunk's normalization is deferred past the next chunk's QK
                # issue; otherwise every cross-engine hop head-of-line
                # blocks the PE.
                pairs = []   # (J, idxA, idxB, n_J)
                ent_of_J = {}
                for J in range(NCH if do_attn else 0):
                    entries = []
                    for c in range(J):
                        for i in range(TPC):
                            entries.append((TPC * c + i, None))
                    for c in range(J):
                        for i in range(TPC):
                            entries.append((NTl + TPC * c + i, None))
                    for i in (1, 0, 3, 2):
                        entries.append((TPC * J + i, ("tri", i ^ 1, i)))
                    for i in range(TPC):
                        entries.append((NTl + TPC * J + i, ("pm", 0)))
                    ent_of_J[J] = entries
                    for p in range(len(entries) // 2):
                        pairs.append((J, 2 * p, 2 * p + 1, len(entries)))

                G = len(pairs)
                DEPTH = depth
                score_tiles = {}
                o_of_J = {}

                def issue_qk(g):
                    J, ia, ib, _n = pairs[g]
                    qb = slice(J * CH, (J + 1) * CH)
                    t0 = ent_of_J[J][ia][0]
                    t1 = ent_of_J[J][ib][0]
                    S = psS.tile([128, 2 * CH], F32, tag="s", name="S")
                    c0 = slice(t0 * 128, t0 * 128 + 128)
                    c1 = slice(t1 * 128, t1 * 128 + 128)
                    nc.tensor.matmul(S[:, 0:CH], kdup[0:64, c0],
                                     qdup[0:64, qb], start=True, stop=True)
                    nc.tensor.matmul(S[:, CH:2 * CH], kdup[64:128, c1],
                                     qdup[64:128, qb], start=True, stop=True)
                    score_tiles[g] = S

                def do_pair(g):
                    J, ia, ib, n = pairs[g]
                    S = score_tiles.pop(g)
                    (t0, mask_a) = ent_of_J[J][ia]
                    (t1, mask_b) = ent_of_J[J][ib]
                    ex = expool.tile([128, 2 * CH], BF16, tag="ex")
                    # one exp over both banks of the pair - amortizes the
                    # fixed ScalarE instruction cost; diagonal pairs are
                    # steeper-tile-first, so cols < 128*tile are all-masked
                    # and the exp skips them (stale ex content there is
                    # zeroed by the tri multiply; buffers are zero-inited)
                    c0 = 128 * mask_a[2] if (mask_a and mask_a[0] == "tri") \
                        else 0
                    nc.scalar.activation(ex[:, c0:], S[:, c0:],
                                         mybir.ActivationFunctionType.Exp,
                                         scale=SCALE)
                    if mask_a is not None and mask_a[0] == "tri":
                        s0 = mask_a[1]
                        fused = (mask_b is not None and mask_b[0] == "tri"
                                 and mask_b[1] == s0 + 1)
                        if fused:
                            nc.vector.tensor_mul(
                                ex[:], ex[:],
                                tri[:, s0:s0 + 2, :].rearrange(
                                    "p t c -> p (t c)"))
                        else:
                            nc.vector.tensor_mul(ex[:, 0:CH], ex[:, 0:CH],
                                                 tri[:, s0, :])
                    elif mask_a is not None:
                        nc.vector.tensor_scalar_mul(ex[:], ex[:], pm[:, :])
                    if (mask_b is not None and mask_b[0] == "tri"
                            and not (mask_a is not None
                                     and mask_a[0] == "tri"
                                     and mask_b[1] == mask_a[1] + 1)):
                        nc.vector.tensor_mul(ex[:, CH:2 * CH],
                                             ex[:, CH:2 * CH],
                                             tri[:, mask_b[1], :])
                    if ia == 0:
                        o_of_J[J] = psO.tile([128, CH], F32, tag="o",
                                             name="o_ps")
                    o_ps = o_of_J[J]
                    nc.tensor.matmul(o_ps[0:H + 1, :], v_nat[:, t0, 0:H + 1],
                                     ex[:, 0:CH],
                                     start=(ia == 0), stop=False)
                    nc.tensor.matmul(o_ps[0:H + 1, :], v_nat[:, t1, 0:H + 1],
                                     ex[:, CH:2 * CH],
                                     start=False, stop=(ib == n - 1))

                def finish_J(J):
                    qb = slice(J * CH, (J + 1) * CH)
                    o_ps = o_of_J.pop(J)
                    recip = wpool.tile([1, CH], F32R, tag="recip")
                    with nc.allow_low_precision(reason="f32r recip for norm"):
                        nc.vector.reciprocal(recip[:], o_ps[H:H + 1, :])
                    o_sb0 = wpool.tile([H, CH], F32, tag="osb0")
                    nc.vector.tensor_copy(o_sb0[:], o_ps[0:H, :])
                    rb_ps = psO.tile([128, CH], F32, tag="o", name="rb_ps")
                    nc.tensor.matmul(rb_ps[0:H, :], ones64[:], recip[:],
                                     start=True, stop=True)
                    nc.vector.tensor_mul(o_stage[:, qb], o_sb0[:],
                                         rb_ps[0:H, :])

                for g in range(min(DEPTH, G)):
                    issue_qk(g)
                pend_fin = None
                for g in range(G):
                    if g + DEPTH < G:
                        issue_qk(g + DEPTH)
                    if pend_fin is not None:
                        finish_J(pend_fin)
                        pend_fin = None
                    J, ia, ib, n = pairs[g]
                    do_pair(g)
                    if ib == n - 1:
                        if defer_fin:
                            pend_fin = J
                        else:
                            finish_J(J)
                if pend_fin is not None:
                    finish_J(pend_fin)
                if do_attn and out_mode == 'batched':
                    nc.sync.dma_start(out=outT_ext.ap()[:, :], in_=o_stage[:])

            if n_loop == 1:
                body()
                if out_mode == 'none':
                    nc.sync.dma_start(out=outT_ext.ap()[:, :], in_=o_stage[:])
            elif unrolled:
                for _i in range(n_loop):
                    body(_i)
            else:
                with tc.For_i(0, n_loop, 1) as iv:
                    body(iv)
                if out_mode == 'none':
                    nc.sync.dma_start(out=outT_ext.ap()[:, :], in_=o_stage[:])

    nc.compile()
    return nc


# ---------------- host-side shard / unshard ----------------

def make_in_maps(x, Wq, Wk, Wv, Tg=T, CH=512):
    import ml_dtypes
    Tl = Tg // 2
    NCH = Tl // CH
    NB = 2 * NCH
    NE = np.asarray(Wq).shape[0] // 128
    x = np.asarray(x)
    w_all = np.concatenate([np.asarray(Wq), np.asarray(Wk), np.asarray(Wv)],
                           axis=1).astype(ml_dtypes.bfloat16)     # [E, 192]
    w_tiled = np.ascontiguousarray(w_all.reshape(NE, 128, 192))

    in_maps = []
    for core in range(N_CORES):
        b, o = core // 2, core % 2
        own = [2 * J + o for J in range(NCH)]
        par = [2 * J + (1 - o) for J in range(NCH)]
        xl = np.concatenate([x[b, g * CH:(g + 1) * CH, :] for g in own + par],
                            axis=0)                               # [Tg, E]
        # xb[blk, p, e*CH + c] = xl[blk*CH + c, e*128 + p]
        xb = xl.reshape(NB, CH, NE, 128).transpose(0, 3, 2, 1)
        xb = np.ascontiguousarray(
            xb.reshape(NB, 128, NE * CH).astype(ml_dtypes.bfloat16))
        pmv = np.full((128, 1), 1.0 if o == 1 else 0.0, np.float32)
        in_maps.append({"xb": xb, "w": w_tiled, "pm": pmv,
                        "ones": np.ones((1, H), np.float32)})
    return in_maps


def unshard_out(results, Tg=T, CH=512):
    Tl = Tg // 2
    NCH = Tl // CH
    out = np.zeros((B, Tg, H), np.float32)
    for core in range(N_CORES):
        b, o = core // 2, core % 2
        outT = np.asarray(results[core]["outT"]).astype(np.float32)
        for J in range(NCH):
            g = 2 * J + o
            out[b, g * CH:(g + 1) * CH, :] = outT[:, J * CH:(J + 1) * CH].T
    return out


_cached_nc = None


def kernel(x, Wq, Wk, Wv):
    global _cached_nc
    if _cached_nc is None:
        _cached_nc = build_attention()
    in_maps = make_in_maps(x, Wq, Wk, Wv)
    res = run_bass_kernel_spmd(_cached_nc, in_maps, core_ids=list(range(N_CORES)))
    return unshard_out(res.results)



# revision 36
# speedup vs baseline: 1.6146x; 1.2357x over previous
"""Distributed causal single-head attention for 8 Trainium2 NeuronCores.

Problem: x [B=4, T=4096, E=1024] f32; Wq/Wk/Wv [E, H=64] f32.
out[b] = softmax(causal(q k^T / sqrt(H))) v,  q/k/v = x[b] @ W.

Sharding: core = (batch b = core//2, parity o = core%2). Each core computes
the output rows of the interleaved 512-row chunks {2J+o : J=0..3} of batch b.
The host ships x[b]^T (bf16, tile-blocked for contiguous DMA) with columns
permuted to [own-chunks | partner-chunks] so all 8 cores run one identical
(SPMD) instruction stream; parity enters only through data (a scalar mask
input). Causal masks are generated on-device.

On this stack, externally-supplied DRAM buffers stream ~10x slower than
SBUF-resident data, so all of x (8 MB bf16/core) is staged into SBUF once
up front on the sync HWDGE queue; the compute body never touches external
DRAM again except the output store. The body itself is balanced across
three ~70 us engine streams (ScalarE exp / PE matmul / DVE evacuations):

  - q/k/v projections as bf16 matmuls (qk-pack [Wq|Wk], kv-pack [Wk|Wv]);
    own-block v is produced directly in natural [k,h] layout (stationary =
    x^T tile, moving = Wv), partner-block v^T goes through packed PE
    transposes (identity matmul, 4 tiles per PSUM bank, one evacuation
    copy per bank),
  - scoresT[k,q] via row-packed K=64 matmul pairs - the q/k row
    duplication places the pair at PE row offsets 0/64, which the
    hardware runs concurrently (inferred tile_position row tiling),
  - exp on ScalarE (PSUM f32 -> SBUF bf16, the throughput floor at
    ~880 ns per [128,512] tile), causal/parity masks multiplied in on DVE,
  - PV as bf16 matmuls with an extra ones-row in v_nat producing softmax
    denominators for free,
  - normalization via f32r reciprocal + K=1 broadcast matmul.

The attention phase is a flat software pipeline over all (chunk, k-tile
pair) steps: score matmuls are issued `depth` pairs ahead of their
exp->mask->PV consumers, score PSUM tiles rotate across three pools (six
banks in flight), and each chunk's normalization is deferred past the
next chunk's issue - the in-order PE otherwise head-of-line blocks on
every cross-engine hop. PSUM tiles in the projection phase rotate across
the same three pools.
"""

import sys

sys.path.insert(0, "/opt/trn_rl_repo")

import numpy as np

import concourse.bass as bass  # noqa: F401
import concourse.tile as tile
from concourse import bacc, mybir
from concourse.bass_utils import run_bass_kernel_spmd

F32 = mybir.dt.float32
F32R = mybir.dt.float32r
BF16 = mybir.dt.bfloat16

B, T, E, H = 4, 4096, 1024, 64
N_CORES = 8
SCALE = float(H) ** -0.5


def build_attention(Eb=E, Tg=T, CH=512, n_loop=1, n_cores=N_CORES,
                    do_proj=True, do_attn=True, do_vnat=True,
                    depth=4, defer_fin=True, ex_bufs=4, unrolled=False,
                    direct_v=True, gp_copies=False, out_mode='batched'):
    """Per-core inputs:
    xb [2*NCH, 128, NE*CH] bf16  (xb[b,p,e*CH+c] = x_local[b*CH+c, e*128+p]),
    w  [NE, 128, 192] bf16       ([Wq | Wk | Wv] row-tiled),
    pm [128, 1] f32, ones [1, H] f32r  ->  outT [H, Tl] bf16.
    """
    Tl = Tg // 2
    TPC = CH // 128          # k-tiles per chunk
    NCH = Tl // CH           # local q-chunks per core
    NTl = Tl // 128          # own k-tiles
    NTg = 2 * NTl            # all k-tiles (own + partner)
    NE = Eb // 128           # contraction tiles
    NB = 2 * NCH             # column blocks (own + partner)
    VW = 128                 # padded v row width (xbar-safe)

    nc = bacc.Bacc("TRN2", target_bir_lowering=False, debug=False,
                   num_devices=n_cores)
    xb_ext = nc.dram_tensor("xb", [NB, 128, NE * CH], BF16, kind="ExternalInput")
    w_ext = nc.dram_tensor("w", [NE, 128, 192], BF16, kind="ExternalInput")
    pm_ext = nc.dram_tensor("pm", [128, 1], F32, kind="ExternalInput")
    ones_ext = nc.dram_tensor("ones", [1, H], F32R, kind="ExternalInput")
    outT_ext = nc.dram_tensor("outT", [H, Tl], BF16, kind="ExternalOutput")

    with tile.TileContext(nc) as tc:
        with (
            tc.tile_pool(name="const", bufs=1) as cpool,
            tc.tile_pool(name="big", bufs=1) as bigpool,
            tc.tile_pool(name="work", bufs=4) as wpool,
            tc.tile_pool(name="exp", bufs=ex_bufs) as expool,
            tc.tile_pool(name="psS", bufs=3, space="PSUM") as psS,
            tc.tile_pool(name="psO", bufs=2, space="PSUM") as psO,
        ):
            # ---- constants ----
            w = cpool.tile([128, NE, 192], BF16, tag="w")
            tri = cpool.tile([128, TPC, CH], BF16, tag="tri")
            pm = cpool.tile([128, 1], F32, tag="pm")
            ones64 = cpool.tile([1, H], F32R, tag="ones64")
            for e in range(NE):
                nc.gpsimd.dma_start(out=w[:, e, :], in_=w_ext.ap()[e])
            nc.gpsimd.dma_start(out=pm[:, :], in_=pm_ext.ap())
            nc.gpsimd.dma_start(out=ones64[:, :], in_=ones_ext.ap())
            # tri slot s holds the mask of diag tile i = s^1 (pairs are
            # emitted steeper-tile-first so the all-masked region is a
            # contiguous prefix the exp can skip; slot-swapping keeps the
            # fused two-tile mask multiply contiguous): mask[i][p, f] =
            # 1.0 if 128*i + p <= f else 0.0
            for i in range(TPC):
                slot = i ^ 1
                nc.gpsimd.memset(tri[:, slot, :], 0.0)
                nc.gpsimd.affine_select(
                    out=tri[:, slot, :], in_=tri[:, slot, :],
                    compare_op=mybir.AluOpType.is_gt, fill=1.0,
                    base=128 * i, pattern=[[-1, CH]], channel_multiplier=1)
            ident = cpool.tile([128, 128], F32, tag="ident")
            from concourse.masks import make_identity
            make_identity(nc, ident[:, :])

            # ---- x resident in SBUF (loaded once, before the loop) ----
            # External-input DRAM in this environment streams ~10x slower
            # than SBUF-resident data, so stage all of x into SBUF up front
            # on the sync HWDGE queue; the compute loop never touches
            # external DRAM again except the output store.
            xsb = cpool.tile([128, NB, NE, CH], BF16, tag="xsb")
            for b in range(NB):
                nc.sync.dma_start(
                    out=xsb[:, b, :, :],
                    in_=xb_ext.ap()[b].rearrange("p (e c) -> p e c", e=NE))

            # persistent across loop iterations; fully rewritten each pass
            qdup = bigpool.tile([128, Tl], BF16, tag="qdup")
            kdup = bigpool.tile([128, Tg], BF16, tag="kdup")
            vT_sb = bigpool.tile([H, Tl], F32, tag="vT_sb")
            vT_own = bigpool.tile([H, Tl], F32, tag="vT_own")
            v_nat = bigpool.tile([128, NTg, VW], BF16, tag="v_nat")
            o_stage = bigpool.tile([H, Tl], BF16, tag="o_stage")
            # zero cols once so the padded 128-wide PV stationary (for FWL)
            # multiplies garbage-free; ones column = softmax denominator
            nc.vector.memset(v_nat[:, :, :], 0.0)
            nc.vector.memset(v_nat[:, :, H:H + 1], 1.0)
            for _i in range(ex_bufs):
                exz = expool.tile([128, 2 * CH], BF16, tag="ex", name="exz")
                nc.vector.memset(exz[:, :], 0.0)

            def body(_iv=0, unroll=1):
                _pp = [0, None]

                def proj_ps(nm):
                    if _pp[0] % 2 == 0:
                        _pp[1] = psS.tile([128, 2 * CH], F32, tag="s",
                                          name=nm)
                        half = _pp[1][:, 0:CH]
                    else:
                        half = _pp[1][:, CH:2 * CH]
                    _pp[0] += 1
                    return half

                # ---- projections ----
                if not do_proj:
                    nc.vector.memset(qdup[:, :], 0.0)
                    nc.vector.memset(kdup[:, :], 0.0)
                    nc.vector.memset(vT_sb[0:64, :], 0.0)
                # blocks in pairs: two qk/kv chains fill the two banks of
                # one PSUM double tile, evacuated with single fused copies
                for b0 in range(0, NB if do_proj else 0, 2):
                    own = b0 < NCH
                    S = psS.tile([128, 2 * CH], F32, tag="s", name="Sp")
                    for h, b in ((0, b0), (1, b0 + 1)):
                        woff = 0 if own else 64  # [Wq|Wk] or [Wk|Wv]
                        for e in range(NE):
                            nc.tensor.matmul(S[:, h * CH:h * CH + CH],
                                             w[:, e, woff:woff + 128],
                                             xsb[:, b, e, :],
                                             start=(e == 0),
                                             stop=(e == NE - 1))
                    cols2 = slice(b0 * CH, (b0 + 2) * CH)
                    if own:
                        nc.vector.tensor_copy(qdup[0:64, cols2], S[0:64, :])
                        nc.vector.tensor_copy(qdup[64:128, cols2], S[0:64, :])
                        nc.vector.tensor_copy(kdup[0:64, cols2], S[64:128, :])
                        nc.vector.tensor_copy(kdup[64:128, cols2],
                                              S[64:128, :])
                        # v in natural [k, h] layout directly: stationary is
                        # the x^T tile (contraction = e), moving is Wv
                        V = psS.tile([128, 2 * CH], F32, tag="s", name="Vp")
                        for h, b in ((0, b0), (1, b0 + 1)):
                            for j in range(TPC):
                                for e in range(NE):
                                    nc.tensor.matmul(
                                        V[:, h * CH + j * 128:
                                          h * CH + j * 128 + H],
                                        xsb[:, b, e, j * 128:(j + 1) * 128],
                                        w[:, e, 128:192],
                                        start=(e == 0), stop=(e == NE - 1))
                        nc.vector.tensor_copy(
                            v_nat[:, b0 * TPC:(b0 + 2) * TPC, 0:H],
                            V[:, 0:2 * CH].rearrange(
                                "p (h t c) -> p (h t) c", h=2, t=TPC)[
                                :, :, 0:H])
                    else:
                        nc.vector.tensor_copy(kdup[0:64, cols2], S[0:64, :])
                        nc.vector.tensor_copy(kdup[64:128, cols2], S[0:64, :])
                        pcols2 = slice((b0 - NCH) * CH, (b0 - NCH + 2) * CH)
                        nc.vector.tensor_copy(vT_sb[0:64, pcols2],
                                              S[64:128, :])

                # v natural (+ones row): per-tile PE transpose [65,128]->[128,65]
                # through PSUM (identity matmul) - far cheaper than the xbar
                # DMA-transpose path, and it pipelines with the projections
                # on the same engine.
                srcs = [(NTl, vT_sb)] if direct_v else [(NTl, vT_sb),
                                                         (0, vT_own)]
                for t0, vsrc in (srcs if do_vnat else []):
                    for t4 in range(NTl // TPC):
                        tps = proj_ps("tps")
                        for j in range(TPC):
                            t = t4 * TPC + j
                            nc.tensor.transpose(
                                tps[:, j * H:(j + 1) * H],
                                vsrc[0:H, t * 128:(t + 1) * 128],
                                ident[0:H, 0:H])
                        nc.vector.tensor_copy(
                            v_nat[:, t0 + t4 * TPC:t0 + (t4 + 1) * TPC, 0:H],
                            tps[:, 0:TPC * H].rearrange("p (t c) -> p t c",
                                                        t=TPC))

                # ---- attention ----
                if not do_attn:
                    nc.sync.dma_start(out=outT_ext.ap()[:, 0:Tl],
                                      in_=qdup[0:H, 0:Tl])
                # Flat software pipeline over all (J, k-tile-pair) steps.
                # The PE is in-order, so QK score matmuls are issued DEPTH
                # pairs ahead of the exp->mask->PV consumer chain, and each
                # chunk's normalization is deferred past the next chunk's QK
                # issue; otherwise every cross-engine hop head-of-line
                # blocks the PE.
                pairs = []   # (J, idxA, idxB, n_J)
                ent_of_J = {}
                for J in range(NCH if do_attn else 0):
                    entries = []
                    for c in range(J):
                        for i in range(TPC):
                            entries.append((TPC * c + i, None))
                    for c in range(J):
                        for i in range(TPC):
                            entries.append((NTl + TPC * c + i, None))
                    for i in (1, 0, 3, 2):
                        entries.append((TPC * J + i, ("tri", i ^ 1, i)))
                    for i in range(TPC):
                        entries.append((NTl + TPC * J + i, ("pm", 0)))
                    ent_of_J[J] = entries
                    for p in range(len(entries) // 2):
                        pairs.append((J, 2 * p, 2 * p + 1, len(entries)))

                G = len(pairs)
                DEPTH = depth
                score_tiles = {}
                o_of_J = {}

                def issue_qk(g):
                    J, ia, ib, _n = pairs[g]
                    t0, mask_a = ent_of_J[J][ia]
                    t1 = ent_of_J[J][ib][0]
                    # steep-diagonal halves skip their all-masked leading
                    # columns (exp never reads them; stale PSUM unread)
                    qs = 128 * mask_a[2] \
                        if (mask_a and mask_a[0] == "tri") else 0
                    qa = slice(J * CH + qs, (J + 1) * CH)
                    qb = slice(J * CH, (J + 1) * CH)
                    S = psS.tile([128, 2 * CH], F32, tag="s", name="S")
                    c0 = slice(t0 * 128, t0 * 128 + 128)
                    c1 = slice(t1 * 128, t1 * 128 + 128)
                    nc.tensor.matmul(S[:, qs:CH], kdup[0:64, c0],
                                     qdup[0:64, qa], start=True, stop=True)
                    nc.tensor.matmul(S[:, CH:2 * CH], kdup[64:128, c1],
                                     qdup[64:128, qb], start=True, stop=True)
                    score_tiles[g] = S

                def do_pair(g):
                    J, ia, ib, n = pairs[g]
                    S = score_tiles.pop(g)
                    (t0, mask_a) = ent_of_J[J][ia]
                    (t1, mask_b) = ent_of_J[J][ib]
                    ex = expool.tile([128, 2 * CH], BF16, tag="ex")
                    # one exp over both banks of the pair - amortizes the
                    # fixed ScalarE instruction cost; diagonal pairs are
                    # steeper-tile-first, so cols < 128*tile are all-masked
                    # and the exp skips them (stale ex content there is
                    # zeroed by the tri multiply; buffers are zero-inited)
                    c0 = 128 * mask_a[2] if (mask_a and mask_a[0] == "tri") \
                        else 0
                    nc.scalar.activation(ex[:, c0:], S[:, c0:],
                                         mybir.ActivationFunctionType.Exp,
                                         scale=SCALE)
                    if mask_a is not None and mask_a[0] == "tri":
                        s0 = mask_a[1]
                        fused = (mask_b is not None and mask_b[0] == "tri"
                                 and mask_b[1] == s0 + 1)
                        if fused:
                            nc.vector.tensor_mul(
                                ex[:], ex[:],
                                tri[:, s0:s0 + 2, :].rearrange(
                                    "p t c -> p (t c)"))
                        else:
                            nc.vector.tensor_mul(ex[:, 0:CH], ex[:, 0:CH],
                                                 tri[:, s0, :])
                    elif mask_a is not None:
                        nc.vector.tensor_scalar_mul(ex[:], ex[:], pm[:, :])
                    if (mask_b is not None and mask_b[0] == "tri"
                            and not (mask_a is not None
                                     and mask_a[0] == "tri"
                                     and mask_b[1] == mask_a[1] + 1)):
                        nc.vector.tensor_mul(ex[:, CH:2 * CH],
                                             ex[:, CH:2 * CH],
                                             tri[:, mask_b[1], :])
                    if ia == 0:
                        o_of_J[J] = psO.tile([128, CH], F32, tag="o",
                                             name="o_ps")
                    o_ps = o_of_J[J]
                    nc.tensor.matmul(o_ps[0:H + 1, :], v_nat[:, t0, 0:H + 1],
                                     ex[:, 0:CH],
                                     start=(ia == 0), stop=False)
                    nc.tensor.matmul(o_ps[0:H + 1, :], v_nat[:, t1, 0:H + 1],
                                     ex[:, CH:2 * CH],
                                     start=False, stop=(ib == n - 1))

                def finish_J(J):
                    qb = slice(J * CH, (J + 1) * CH)
                    o_ps = o_of_J.pop(J)
                    recip = wpool.tile([1, CH], F32R, tag="recip")
                    with nc.allow_low_precision(reason="f32r recip for norm"):
                        nc.vector.reciprocal(recip[:], o_ps[H:H + 1, :])
                    o_sb0 = wpool.tile([H, CH], F32, tag="osb0")
                    nc.vector.tensor_copy(o_sb0[:], o_ps[0:H, :])
                    rb_ps = psO.tile([128, CH], F32, tag="o", name="rb_ps")
                    nc.tensor.matmul(rb_ps[0:H, :], ones64[:], recip[:],
                                     start=True, stop=True)
                    nc.vector.tensor_mul(o_stage[:, qb], o_sb0[:],
                                         rb_ps[0:H, :])

                for g in range(min(DEPTH, G)):
                    issue_qk(g)
                pend_fin = None
                for g in range(G):
                    if g + DEPTH < G:
                        issue_qk(g + DEPTH)
                    if pend_fin is not None:
                        finish_J(pend_fin)
                        pend_fin = None
                    J, ia, ib, n = pairs[g]
                    do_pair(g)
                    if ib == n - 1:
                        if defer_fin:
                            pend_fin = J
                        else:
                            finish_J(J)
                if pend_fin is not None:
                    finish_J(pend_fin)
                if do_attn and out_mode == 'batched':
                    nc.sync.dma_start(out=outT_ext.ap()[:, :], in_=o_stage[:])

            if n_loop == 1:
                body()
                if out_mode == 'none':
                    nc.sync.dma_start(out=outT_ext.ap()[:, :], in_=o_stage[:])
            elif unrolled:
                for _i in range(n_loop):
                    body(_i)
            else:
                with tc.For_i(0, n_loop, 1) as iv:
                    body(iv)
                if out_mode == 'none':
                    nc.sync.dma_start(out=outT_ext.ap()[:, :], in_=o_stage[:])

    nc.compile()
    return nc


# ---------------- host-side shard / unshard ----------------

def make_in_maps(x, Wq, Wk, Wv, Tg=T, CH=512):
    import ml_dtypes
    Tl = Tg // 2
    NCH = Tl // CH
    NB = 2 * NCH
    NE = np.asarray(Wq).shape[0] // 128
    x = np.asarray(x)
    w_all = np.concatenate([np.asarray(Wq), np.asarray(Wk), np.asarray(Wv)],
                           axis=1).astype(ml_dtypes.bfloat16)     # [E, 192]
    w_tiled = np.ascontiguousarray(w_all.reshape(NE, 128, 192))

    in_maps = []
    for core in range(N_CORES):
        b, o = core // 2, core % 2
        own = [2 * J + o for J in range(NCH)]
        par = [2 * J + (1 - o) for J in range(NCH)]
        xl = np.concatenate([x[b, g * CH:(g + 1) * CH, :] for g in own + par],
                            axis=0)                               # [Tg, E]
        # xb[blk, p, e*CH + c] = xl[blk*CH + c, e*128 + p]
        xb = xl.reshape(NB, CH, NE, 128).transpose(0, 3, 2, 1)
        xb = np.ascontiguousarray(
            xb.reshape(NB, 128, NE * CH).astype(ml_dtypes.bfloat16))
        pmv = np.full((128, 1), 1.0 if o == 1 else 0.0, np.float32)
        in_maps.append({"xb": xb, "w": w_tiled, "pm": pmv,
                        "ones": np.ones((1, H), np.float32)})
    return in_maps


def unshard_out(results, Tg=T, CH=512):
    Tl = Tg // 2
    NCH = Tl // CH
    out = np.zeros((B, Tg, H), np.float32)
    for core in range(N_CORES):
        b, o = core // 2, core % 2
        outT = np.asarray(results[core]["outT"]).astype(np.float32)
        for J in range(NCH):
            g = 2 * J + o
            out[b, g * CH:(g + 1) * CH, :] = outT[:, J * CH:(J + 1) * CH].T
    return out


_cached_nc = None


def kernel(x, Wq, Wk, Wv):
    global _cached_nc
    if _cached_nc is None:
        _cached_nc = build_attention()
    in_maps = make_in_maps(x, Wq, Wk, Wv)
    res = run_bass_kernel_spmd(_cached_nc, in_maps, core_ids=list(range(N_CORES)))
    return unshard_out(res.results)

